# revision 13
# baseline (speedup 1.0000x reference)
"""DEMONetHashGraph Trainium2 kernel — 8-core data-parallel GNN.

Strategy:
- Fold the multi-hash einsum+concat+Wp into a single [512,512] weight on host:
  hashed @ Wp == agg @ (hstack(Hm) @ Wp).
- Shard nodes (and their outgoing edges) across 8 cores by contiguous range.
- Per layer: segment-mean via one-hot matmuls in PSUM over src-sorted edge
  tiles (gathered neighbor rows via indirect DMA, bf16), then two fp32r dense
  matmuls (hash path on agg, self path on h) + bias + ELU.
- One bf16 AllGather of h1 between layers; tiny AllReduce for per-graph pools.
"""

import sys

for _p in ("/opt/trn_rl_repo", "/root/.axon_site/_ro/trn_rl_repo"):
    if _p not in sys.path:
        sys.path.insert(0, _p)

import time
from contextlib import ExitStack

import ml_dtypes
import numpy as np

import concourse.bass as bass
import concourse.mybir as mybir
import concourse.tile as tile
from concourse import bacc
from concourse.masks import make_identity

# problem constants (hardcoded per spec)
N_NODES = 50000
N_EDGES = 800000
D = 512
NUM_GRAPHS = 64
NUM_CLASSES = 10
NC = 8
SHARD = N_NODES // NC  # 6250
BN = 128
NB = (SHARD + BN - 1) // BN  # 49
SHARD_PAD = NB * BN  # 6272
NPAD = NC * SHARD_PAD  # 50176
SPLIT = 32768

f32 = mybir.dt.float32
f32r = mybir.dt.float32r
bf16 = mybir.dt.bfloat16
i32 = mybir.dt.int32
BF = ml_dtypes.bfloat16


def _preprocess(x, edge_index, batch, Hm1, Wp1, Ws1, b1, Hm2, Wp2, Ws2, b2, Wc, bc):
    x = np.asarray(x, np.float32)
    src = np.asarray(edge_index[0], np.int64)
    dst = np.asarray(edge_index[1], np.int64)
    batch = np.asarray(batch, np.int64)

    deg = np.bincount(src, minlength=N_NODES)
    iso = np.where(deg == 0)[0]
    if iso.size:
        src = np.concatenate([src, iso])
        dst = np.concatenate([dst, iso])

    order = np.argsort(src, kind="stable")
    src_s = src[order]
    dst_s = dst[order]
    # remap dst to padded (per-core) row coordinates
    dst_pad = (dst_s // SHARD) * SHARD_PAD + (dst_s % SHARD)

    # per-(core, block) edge ranges
    blk_starts = []
    for c in range(NC):
        for b in range(NB):
            blk_starts.append(c * SHARD + b * BN)
    blk_starts.append(N_NODES)
    bounds = np.searchsorted(src_s, np.array(blk_starts))

    # split each block's edges by gather-table half (int16 index limit)
    lo_mask = dst_pad < SPLIT
    cnt_lo = np.zeros((NC, NB), np.int64)
    cnt_hi = np.zeros((NC, NB), np.int64)
    for c in range(NC):
        for b in range(NB):
            k = c * NB + b
            m = lo_mask[bounds[k] : bounds[k + 1]]
            cnt_lo[c, b] = int(m.sum())
            cnt_hi[c, b] = int((~m).sum())
    T_lo = max(1, int(np.max((cnt_lo + BN - 1) // BN)))
    T_hi = max(1, int(np.max((cnt_hi + BN - 1) // BN)))
    T = T_lo + T_hi

    slot_arr = np.full((NC, BN, NB * T), -1.0, np.float32)
    ilo = np.zeros((NC, 16, NB * T_lo * 8), np.int16)
    ihi = np.zeros((NC, 16, NB * T_hi * 8), np.int16)
    for c in range(NC):
        for b in range(NB):
            k = c * NB + b
            e0, e1 = bounds[k], bounds[k + 1]
            m = lo_mask[e0:e1]
            d_blk = dst_pad[e0:e1]
            s_blk = (src_s[e0:e1] - (c * SHARD + b * BN)).astype(np.float32)
            for half, sel, base_t, tt in (
                (0, m, 0, T_lo),
                (1, ~m, T_lo, T_hi),
            ):
                d = d_blk[sel] - (SPLIT if half else 0)
                s = s_blk[sel]
                n = len(d)
                if n:
                    j = np.arange(n)
                    slot_arr[c, j % BN, b * T + base_t + j // BN] = s
                    arr = ilo if half == 0 else ihi
                    arr[c, j % 16, b * tt * 8 + j // 16] = d.astype(np.int16)

    # inv deg / batch slots per (core, partition, block)
    node_idx = (
        np.arange(NC)[:, None, None] * SHARD
        + np.arange(NB)[None, None, :] * BN
        + np.arange(BN)[None, :, None]
    )  # [NC, BN, NB]
    valid = node_idx < (np.arange(NC)[:, None, None] + 1) * SHARD
    node_clip = np.minimum(node_idx, N_NODES - 1)
    invdeg = np.where(valid, 1.0 / np.maximum(deg[node_clip], 1), 1.0).astype(
        np.float32
    )
    bslot = np.where(valid, batch[node_clip].astype(np.float32), -1.0).astype(
        np.float32
    )

    cnt_g = np.bincount(batch, minlength=NUM_GRAPHS).astype(np.float32)
    invcnt = np.broadcast_to(
        (1.0 / np.maximum(cnt_g, 1.0))[None, :], (BN, NUM_GRAPHS)
    ).copy()

    # padded x (gather table, bf16) and per-core fp32 shards
    x_pad = np.zeros((NPAD, D), np.float32)
    x_pad_view = x_pad.reshape(NC, SHARD_PAD, D)
    x_pad_view[:, :SHARD, :] = x.reshape(NC, SHARD, D)
    x_bf = x_pad.astype(BF)
    x_shards = [np.ascontiguousarray(x_pad_view[c]) for c in range(NC)]

    # folded weights
    def fold(Hm, Wp):
        Hcat = np.concatenate([np.asarray(Hm, np.float32)[k] for k in range(4)], axis=1)
        return (Hcat @ np.asarray(Wp, np.float32)).astype(np.float32)

    w = dict(
        w1a=fold(Hm1, Wp1),
        wsa=np.asarray(Ws1, np.float32),
        w1b=fold(Hm2, Wp2),
        wsb=np.asarray(Ws2, np.float32).astype(BF),
        wc=np.asarray(Wc, np.float32),
        b1=np.asarray(b1, np.float32).reshape(1, D),
        b2=np.asarray(b2, np.float32).reshape(1, D),
        bc=np.asarray(bc, np.float32).reshape(1, NUM_CLASSES),
        ones=np.ones((1, BN), np.float32),
    )
    return dict(
        T=T,
        T_lo=T_lo,
        T_hi=T_hi,
        ilo=ilo,
        ihi=ihi,
        slot=slot_arr.astype(BF),
        invdeg=invdeg,
        bslot=bslot,
        invcnt=invcnt,
        x_bf=x_bf,
        x_shards=x_shards,
        w=w,
    )


def _build(T, T_lo, T_hi, reps=1, ablate=()):
    nc = bacc.Bacc(
        "TRN2",
        target_bir_lowering=False,
        debug=False,
        num_devices=NC,
        num_swdge_queues=2,
    )

    ein = dict(kind="ExternalInput")
    xg_d = nc.dram_tensor("xg", [NPAD, D], bf16, **ein)
    xs_d = nc.dram_tensor("xs", [SHARD_PAD, D], f32, **ein)
    ilo_d = nc.dram_tensor("ilo", [16, NB * T_lo * 8], mybir.dt.int16, **ein)
    ihi_d = nc.dram_tensor("ihi", [16, NB * T_hi * 8], mybir.dt.int16, **ein)
    slot_d = nc.dram_tensor("slot", [BN, NB * T], bf16, **ein)
    invdeg_d = nc.dram_tensor("invdeg", [BN, NB], f32, **ein)
    bslot_d = nc.dram_tensor("bslot", [BN, NB], f32, **ein)
    invcnt_d = nc.dram_tensor("invcnt", [BN, NUM_GRAPHS], f32, **ein)
    w1a_d = nc.dram_tensor("w1a", [D, D], f32r, **ein)
    wsa_d = nc.dram_tensor("wsa", [D, D], f32r, **ein)
    w1b_d = nc.dram_tensor("w1b", [D, D], f32r, **ein)
    wsb_d = nc.dram_tensor("wsb", [D, D], bf16, **ein)
    wc_d = nc.dram_tensor("wc", [D, NUM_CLASSES], f32r, **ein)
    b1_d = nc.dram_tensor("b1", [1, D], f32r, **ein)
    b2_d = nc.dram_tensor("b2", [1, D], f32r, **ein)
    bc_d = nc.dram_tensor("bc", [1, NUM_CLASSES], f32r, **ein)
    ones_d = nc.dram_tensor("ones", [1, BN], f32r, **ein)
    out_d = nc.dram_tensor("out", [NUM_GRAPHS, NUM_CLASSES], f32, kind="ExternalOutput")

    with tile.TileContext(nc) as tc, ExitStack() as ctx:
        const = ctx.enter_context(tc.tile_pool(name="const", bufs=1))
        dram = ctx.enter_context(tc.tile_pool(name="dram", bufs=1, space="DRAM"))
        gpool = ctx.enter_context(tc.tile_pool(name="gpool", bufs=3))
        spool = ctx.enter_context(tc.tile_pool(name="spool", bufs=2))
        work = ctx.enter_context(tc.tile_pool(name="work", bufs=2))
        hpool = ctx.enter_context(tc.tile_pool(name="hpool", bufs=3))
        ps_nsum = ctx.enter_context(tc.tile_pool(name="ps_nsum", bufs=2, space="PSUM"))
        ps_tr = ctx.enter_context(tc.tile_pool(name="ps_tr", bufs=2, space="PSUM"))
        ps_dense = ctx.enter_context(
            tc.tile_pool(name="ps_dense", bufs=2, space="PSUM")
        )
        ps_pool = ctx.enter_context(tc.tile_pool(name="ps_pool", bufs=1, space="PSUM"))
        ps_fin = ctx.enter_context(tc.tile_pool(name="ps_fin", bufs=1, space="PSUM"))

        # ---- constants / loads ----
        ident = const.tile([BN, BN], f32)
        make_identity(nc, ident[:])
        iota_i = const.tile([BN, BN], i32)
        nc.gpsimd.iota(iota_i[:], pattern=[[1, BN]], base=0, channel_multiplier=0)
        iota_bf = const.tile([BN, BN], bf16)
        nc.vector.tensor_copy(iota_bf[:], iota_i[:])
        iota_g = const.tile([BN, NUM_GRAPHS], f32)
        nc.vector.tensor_copy(iota_g[:], iota_i[:, :NUM_GRAPHS])

        ilo_sb = const.tile([BN, NB * T_lo * 8], mybir.dt.int16)
        ihi_sb = const.tile([BN, NB * T_hi * 8], mybir.dt.int16)
        for rep8 in range(8):
            nc.sync.dma_start(ilo_sb[rep8 * 16 : (rep8 + 1) * 16, :], ilo_d[:, :])
            nc.sync.dma_start(ihi_sb[rep8 * 16 : (rep8 + 1) * 16, :], ihi_d[:, :])
        slot_sb = const.tile([BN, NB * T], bf16)
        nc.sync.dma_start(slot_sb[:], slot_d[:, :])
        invdeg_sb = const.tile([BN, NB], f32)
        nc.sync.dma_start(invdeg_sb[:], invdeg_d[:, :])
        bslot_sb = const.tile([BN, NB], f32)
        nc.sync.dma_start(bslot_sb[:], bslot_d[:, :])
        invcnt_sb = const.tile([BN, NUM_GRAPHS], f32)
        nc.sync.dma_start(invcnt_sb[:], invcnt_d[:, :])

        def load_w(dram_t):
            t = const.tile([BN, 4, D], f32r, name=f"w_{dram_t.name}")
            nc.sync.dma_start(t[:], dram_t[:, :].rearrange("(ks kp) n -> kp ks n", kp=BN))
            return t

        w1a_sb = load_w(w1a_d)
        wsa_sb = load_w(wsa_d)
        w1b_sb = load_w(w1b_d)
        wsb_sb = const.tile([BN, 4, D], bf16)
        nc.sync.dma_start(
            wsb_sb[:], wsb_d[:, :].rearrange("(ks kp) n -> kp ks n", kp=BN)
        )
        wc_sb = const.tile([BN, 4, NUM_CLASSES], f32r)
        nc.sync.dma_start(wc_sb[:], wc_d[:, :].rearrange("(ks kp) n -> kp ks n", kp=BN))
        b1_sb = const.tile([1, D], f32r)
        nc.sync.dma_start(b1_sb[:], b1_d[:, :])
        b2_sb = const.tile([1, D], f32r)
        nc.sync.dma_start(b2_sb[:], b2_d[:, :])
        bc_sb = const.tile([1, NUM_CLASSES], f32r)
        nc.sync.dma_start(bc_sb[:], bc_d[:, :])
        ones_sb = const.tile([1, BN], f32r)
        nc.sync.dma_start(ones_sb[:], ones_d[:, :])

        # ---- internal DRAM ----
        h1s = dram.tile([SHARD_PAD, D], bf16)  # layer-1 out shard (bf16, AG input)
        hT1 = const.tile([BN, 4, SHARD_PAD], bf16)  # resident transposed h1
        gin = dram.tile([BN, 4 * NUM_GRAPHS], f32)
        gout = dram.tile([BN, 4 * NUM_GRAPHS], f32, addr_space="Shared")

        gacc = const.tile([BN, 4 * NUM_GRAPHS], f32)

        def layer(li, table_bf, self_f32, w1_sb, ws_sb, bias_sb):
            for b in range(NB):
                # gather neighbor rows (bf16)
                g = gpool.tile([BN, T, D], bf16, name="g")
                nc.gpsimd.dma_gather(
                    g[:, :T_lo, :],
                    table_bf[:SPLIT, :],
                    ilo_sb[:, b * T_lo * 8 : (b + 1) * T_lo * 8],
                    BN * T_lo,
                    BN * T_lo,
                    D,
                    single_packet=False,
                    queue_num=0,
                )
                nc.gpsimd.dma_gather(
                    g[:, T_lo:, :],
                    table_bf[SPLIT:, :],
                    ihi_sb[:, b * T_hi * 8 : (b + 1) * T_hi * 8],
                    BN * T_hi,
                    BN * T_hi,
                    D,
                    single_packet=False,
                    queue_num=1,
                )
                # one-hot selection matrices for all T edge tiles
                s_t = spool.tile([BN, T, BN], bf16, name="s_t")
                nc.vector.tensor_tensor(
                    out=s_t[:],
                    in0=slot_sb[:, b * T : (b + 1) * T, None].to_broadcast([BN, T, BN]),
                    in1=iota_bf[:, None, :].to_broadcast([BN, T, BN]),
                    op=mybir.AluOpType.is_equal,
                )
                # segment-sum into PSUM
                ps = ps_nsum.tile([BN, D], f32, name="ps")
                if "edgemm" not in ablate:
                    for t in range(T):
                        nc.tensor.matmul(
                            ps[:],
                            lhsT=s_t[:, t, :],
                            rhs=g[:, t, :],
                            start=(t == 0),
                            stop=(t == T - 1),
                        )
                else:
                    nc.tensor.matmul(
                        ps[:], lhsT=s_t[:, 0, :], rhs=g[:, 0, :], start=True, stop=True
                    )
                # mean
                agg = work.tile([BN, D], f32, name="agg")
                nc.vector.tensor_scalar_mul(agg[:], ps[:], invdeg_sb[:, b : b + 1])
                # transpose agg -> [feat, node] fp32r (4 PE transposes, 1 copy)
                aggT = work.tile([BN, 4, BN], f32r, name="aggT")
                pt = ps_tr.tile([BN, D], f32, name="pt", tag="pt")
                for k in range(4):
                    nc.tensor.transpose(
                        pt[:, k * BN : (k + 1) * BN], agg[:, k * BN : (k + 1) * BN], ident[:]
                    )
                nc.vector.tensor_copy(
                    aggT[:], pt[:].rearrange("p (k n) -> p k n", n=BN)
                )
                if li == 0:
                    # self rows from x (fp32) + transpose
                    hb = work.tile([BN, D], f32, name="hb")
                    nc.sync.dma_start(hb[:], self_f32[b * BN : (b + 1) * BN, :])
                    hbT = work.tile([BN, 4, BN], f32r, name="hbT")
                    pt2 = ps_tr.tile([BN, D], f32, name="pt2", tag="pt")
                    for k in range(4):
                        nc.tensor.transpose(
                            pt2[:, k * BN : (k + 1) * BN],
                            hb[:, k * BN : (k + 1) * BN],
                            ident[:],
                        )
                    nc.vector.tensor_copy(
                        hbT[:], pt2[:].rearrange("p (k n) -> p k n", n=BN)
                    )
                else:
                    hbT = hT1[:, :, b * BN : (b + 1) * BN]
                # dense: out = agg @ W1 + h @ Ws + bias
                po = ps_dense.tile([BN, D], f32, name="po")
                nc.tensor.matmul(
                    po[:], lhsT=ones_sb[:, :], rhs=bias_sb[:, :], start=True, stop=False
                )
                for k in range(4):
                    nc.tensor.matmul(
                        po[:],
                        lhsT=aggT[:, k, :],
                        rhs=w1_sb[:, k, :],
                        start=False,
                        stop=False,
                    )
                for k in range(4):
                    nc.tensor.matmul(
                        po[:],
                        lhsT=hbT[:, k, :],
                        rhs=ws_sb[:, k, :],
                        start=False,
                        stop=(k == 3),
                    )
                # ELU: max(x,0)-1 + exp(min(x,0))
                r = work.tile([BN, D], f32, name="r")
                nc.vector.tensor_scalar(
                    r[:], po[:], 0.0, -1.0, mybir.AluOpType.max, mybir.AluOpType.add
                )
                nmin = work.tile([BN, D], f32, name="nmin")
                nc.vector.tensor_scalar_min(nmin[:], po[:], 0.0)
                e = work.tile([BN, D], f32, name="e")
                nc.scalar.activation(e[:], nmin[:], mybir.ActivationFunctionType.Exp)
                h = hpool.tile([BN, D], f32r, name="h")
                nc.vector.tensor_add(h[:], r[:], e[:])

                if li == 0:
                    h_bf = work.tile([BN, D], bf16, name="h_bf")
                    nc.vector.tensor_copy(h_bf[:], h[:].bitcast(f32))
                    nc.sync.dma_start(h1s[b * BN : (b + 1) * BN, :], h_bf[:])
                    pt3 = ps_tr.tile([BN, D], f32, name="pt3", tag="pt")
                    for k in range(4):
                        nc.tensor.transpose(
                            pt3[:, k * BN : (k + 1) * BN],
                            h[:, k * BN : (k + 1) * BN].bitcast(f32),
                            ident[:],
                        )
                    nc.vector.tensor_copy(
                        hT1[:, :, b * BN : (b + 1) * BN],
                        pt3[:].rearrange("p (k n) -> p k n", n=BN),
                    )
                else:
                    # per-graph pooling: gT += h.T @ onehot(batch)
                    bm = spool.tile([BN, NUM_GRAPHS], f32r, name="bm")
                    nc.vector.tensor_tensor(
                        out=bm[:],
                        in0=bslot_sb[:, b : b + 1].to_broadcast([BN, NUM_GRAPHS]),
                        in1=iota_g[:],
                        op=mybir.AluOpType.is_equal,
                    )
                    pg = ps_pool.tile([BN, 4 * NUM_GRAPHS], f32, name="pg")
                    for k in range(4):
                        nc.tensor.matmul(
                            pg[:, k * NUM_GRAPHS : (k + 1) * NUM_GRAPHS],
                            lhsT=h[:, k * BN : (k + 1) * BN],
                            rhs=bm[:],
                            start=True,
                            stop=True,
                        )
                    if b == 0:
                        nc.vector.tensor_copy(gacc[:], pg[:])
                    else:
                        nc.vector.tensor_add(gacc[:], gacc[:], pg[:])

        for _rep in range(reps):
            h1f = dram.tile(
                [NPAD, D], bf16, addr_space="Shared", name=f"h1f_{_rep}"
            )  # AG output
            layer(0, xg_d, xs_d, w1a_sb, wsa_sb, b1_sb)
            nc.gpsimd.collective_compute(
                "AllGather",
                mybir.AluOpType.bypass,
                replica_groups=[list(range(NC))],
                ins=[h1s[:, :]],
                outs=[h1f[:, :]],
            )
            layer(1, xg_d if "xgonly" in ablate else h1f, None, w1b_sb, wsb_sb, b2_sb)

        # pooled sums all-reduce
        nc.sync.dma_start(gin[:, :], gacc[:])
        nc.gpsimd.collective_compute(
            "AllReduce",
            mybir.AluOpType.add,
            replica_groups=[list(range(NC))],
            ins=[gin[:, :]],
            outs=[gout[:, :]],
        )
        gsum = const.tile([BN, 4, NUM_GRAPHS], f32r)
        gs_raw = const.tile([BN, 4 * NUM_GRAPHS], f32)
        nc.sync.dma_start(gs_raw[:], gout[:, :])
        nc.vector.tensor_tensor(
            out=gsum[:],
            in0=gs_raw[:].rearrange("p (k g) -> p k g", g=NUM_GRAPHS),
            in1=invcnt_sb[:, None, :].to_broadcast([BN, 4, NUM_GRAPHS]),
            op=mybir.AluOpType.mult,
        )
        pf = ps_fin.tile([BN, NUM_CLASSES], f32)
        nc.tensor.matmul(
            pf[:NUM_GRAPHS, :],
            lhsT=ones_sb[:, :NUM_GRAPHS],
            rhs=bc_sb[:, :],
            start=True,
            stop=False,
        )
        for k in range(4):
            nc.tensor.matmul(
                pf[:NUM_GRAPHS, :],
                lhsT=gsum[:, k, :],
                rhs=wc_sb[:, k, :],
                start=False,
                stop=(k == 3),
            )
        o = const.tile([NUM_GRAPHS, NUM_CLASSES], f32)
        nc.vector.tensor_copy(o[:], pf[:NUM_GRAPHS, :])
        nc.sync.dma_start(out_d[:, :], o[:])

    nc.compile()
    return nc


def _make_in_maps(pre):
    w = pre["w"]
    in_maps = []
    for c in range(NC):
        in_maps.append(
            {
                "xg": pre["x_bf"],
                "xs": pre["x_shards"][c],
                "ilo": np.ascontiguousarray(pre["ilo"][c]),
                "ihi": np.ascontiguousarray(pre["ihi"][c]),
                "slot": np.ascontiguousarray(pre["slot"][c]),
                "invdeg": np.ascontiguousarray(pre["invdeg"][c]),
                "bslot": np.ascontiguousarray(pre["bslot"][c]),
                "invcnt": pre["invcnt"],
                "w1a": w["w1a"],
                "wsa": w["wsa"],
                "w1b": w["w1b"],
                "wsb": w["wsb"],
                "wc": w["wc"],
                "b1": w["b1"],
                "b2": w["b2"],
                "bc": w["bc"],
                "ones": w["ones"],
            }
        )
    return in_maps


def _run_spmd(nc, in_maps, repeats=1):
    """Execute on 8 cores via PJRT (axon). Returns (out_core0, exec_times_s)."""
    import jax
    import jax.numpy as jnp  # noqa: F401
    from jax.sharding import Mesh, PartitionSpec, NamedSharding
    from jax.experimental.shard_map import shard_map

    import concourse.mybir as mb
    from concourse.bass2jax import (
        _bass_exec_p,
        install_neuronx_cc_hook,
        partition_id_tensor,
    )

    install_neuronx_cc_hook()
    partition_name = nc.partition_id_tensor.name if nc.partition_id_tensor else None

    in_names, out_names, out_avals, zero_outs = [], [], [], []
    for alloc in nc.m.functions[0].allocations:
        if not isinstance(alloc, mb.MemoryLocationSet):
            continue
        name = alloc.memorylocations[0].name
        if alloc.kind == "ExternalInput":
            if name != partition_name:
                in_names.append(name)
        elif alloc.kind == "ExternalOutput":
            shape = tuple(alloc.tensor_shape)
            dtype = mb.dt.np(alloc.dtype)
            out_names.append(name)
            out_avals.append(jax.core.ShapedArray(shape, dtype))
            zero_outs.append(np.zeros(shape, dtype))
    n_params = len(in_names)
    n_outs = len(out_avals)
    all_in_names = list(in_names) + out_names
    if partition_name is not None:
        all_in_names.append(partition_name)
    donate = tuple(range(n_params, n_params + n_outs))

    def _body(*args):
        operands = list(args)
        if partition_name is not None:
            operands.append(partition_id_tensor())
        outs = _bass_exec_p.bind(
            *operands,
            out_avals=tuple(out_avals),
            in_names=tuple(all_in_names),
            out_names=tuple(out_names),
            lowering_input_output_aliases=(),
            sim_require_finite=True,
            sim_require_nnan=True,
            nc=nc,
        )
        return tuple(outs)

    devices = jax.devices()[:NC]
    mesh = Mesh(np.asarray(devices), ("core",))
    in_specs = (PartitionSpec("core"),) * (n_params + n_outs)
    out_specs = (PartitionSpec("core"),) * len(out_names)
    sharded = jax.jit(
        shard_map(
            _body, mesh=mesh, in_specs=in_specs, out_specs=out_specs, check_rep=False
        ),
        donate_argnums=donate,
        keep_unused=True,
    )
    concat_in = [
        np.concatenate([np.asarray(in_maps[c][nm]) for c in range(NC)], axis=0)
        for nm in in_names
    ]
    shard_spec = NamedSharding(mesh, PartitionSpec("core"))
    concat_in_dev = [jax.device_put(a, shard_spec) for a in concat_in]

    def one_exec():
        zeros = [
            jax.device_put(
                np.zeros((NC * z.shape[0], *z.shape[1:]), z.dtype), shard_spec
            )
            for z in zero_outs
        ]
        t0 = time.perf_counter()
        out_arrs = sharded(*concat_in_dev, *zeros)
        jax.block_until_ready(out_arrs)
        return time.perf_counter() - t0, out_arrs

    times = []
    out_arrs = None
    for _ in range(max(1, repeats)):
        dt_s, out_arrs = one_exec()
        times.append(dt_s)

    outs0 = {
        name: np.asarray(out_arrs[i]).reshape(NC, *out_avals[i].shape)[0]
        for i, name in enumerate(out_names)
    }
    return outs0, times


_CACHE = {}


def _get_compiled(pre, reps=1, ablate=()):
    key = (pre["T"], pre["T_lo"], pre["T_hi"], reps, tuple(ablate))
    if key not in _CACHE:
        _CACHE[key] = _build(pre["T"], pre["T_lo"], pre["T_hi"], reps, ablate)
    return _CACHE[key]


def kernel(**inputs) -> np.ndarray:
    pre = _preprocess(**inputs)
    nc = _get_compiled(pre)
    outs, _ = _run_spmd(nc, _make_in_maps(pre), repeats=1)
    return outs["out"].astype(np.float32)


def kernel_timed(inputs, repeats=5, reps=1, ablate=()):
    pre = _preprocess(**inputs)
    nc = _get_compiled(pre, reps, ablate)
    outs, times = _run_spmd(nc, _make_in_maps(pre), repeats=repeats)
    return outs["out"].astype(np.float32), times
